# revision 8
# baseline (speedup 1.0000x reference)
"""Trainium2 Bass kernel for nn_EventSplitter (edge-restricted graph transformer).

kernel(**inputs) takes the FULL unsharded numpy inputs (as produced by
reference.setup_inputs()) and returns the FULL [E, 1] float32 output.

Sharding (8 NeuronCores, one SPMD program):
  - Nodes padded to n_cores * B * 128; core c owns nodes [c*B*128, (c+1)*B*128).
  - Edges assigned to the core owning their dst, sorted by dst; every
    (core, dst-block) run padded to K chunks of 128 edges, so the
    chunk->block map (b = t // K) is a compile-time constant shared by all
    cores (SPMD requires one program).
  - h (residual stream) stays per-core fp32 in SBUF. Each layer: compute
    q (local) and packed [k|v] fp16 rows, AllGather the [k|v] table, then per
    128-edge chunk do a [128,1] indirect-DMA row gather, one-hot S/S^T
    matmuls for q_dst expansion and the segment softmax/sum (PSUM
    accumulation per dst-block), then the node-side Wo/LN/FFN updates.
  - Softmax skips max-subtraction (logits are O(1) here); den division is
    applied at the node level as a / (den + 1e-16), matching the reference.
  - Edge head: u = h3 @ W_e1[:192] + b_e1 per node, AllGathered and row
    gathered per edge; dst part expanded via S^T; z2/z3 via PE.
"""

import math

import numpy as np

import concourse.bass as bass
import concourse.tile as tile
import concourse.mybir as mybir
from concourse.masks import make_identity

try:
    from waitsplit import split_excess_waits
except ImportError:  # self-contained fallback
    def split_excess_waits(nc, max_waits=1):
        ctr = [0]
        for f in nc.m.functions:
            for bb in f.blocks:
                out = []
                changed = False
                for ins in bb.instructions:
                    si = ins.sync_info
                    if si is not None and len(si.on_wait) > max_waits:
                        waits = list(si.on_wait)
                        keep, extra = waits[-max_waits:], waits[:-max_waits]
                        for i in range(0, len(extra), max_waits):
                            chunk = extra[i:i + max_waits]
                            nop = mybir.InstNoOp(name=f"wsplit-{ctr[0]}", ins=[], outs=[])
                            ctr[0] += 1
                            nop.engine = ins.engine
                            nop.sync_info = mybir.SyncInfo(on_wait=list(chunk), on_update=[])
                            out.append(nop)
                        ins.sync_info = mybir.SyncInfo(on_wait=list(keep),
                                                       on_update=list(si.on_update))
                        changed = True
                    out.append(ins)
                if changed:
                    bb.instructions[:] = out
        return nc

P = 128
dt = mybir.dt

HID = 192
HEADS = 4
DH = 48
L = 3
FFN = 384
EA = 4
GP = 3
SP = 3
EP = 6
G_TBL = 512
XS = 7          # x(4) + splitter_probs(3)
H2 = 2 * HID    # 384
Z2 = 96


# ----------------------------------------------------------------------------
# host-side sharding / index prep
# ----------------------------------------------------------------------------

def _host_prep(x, edge_index, edge_attr, batch, group_ptr, time_group_ids,
               group_probs, splitter_probs, endpoint_preds, n_cores):
    N = x.shape[0]
    E = edge_index.shape[1]
    B = int(math.ceil(N / (n_cores * P)))     # dst-blocks per core
    NLp = B * P                               # local nodes per core (padded)
    Np = NLp * n_cores

    src = np.asarray(edge_index[0], np.int64)
    dst = np.asarray(edge_index[1], np.int64)

    gids = np.clip(np.asarray(group_ptr)[np.asarray(batch)] + np.asarray(time_group_ids),
                   0, group_probs.shape[0] - 1).astype(np.int64)

    n_blocks_total = Np // P
    blk_of_edge = dst // P
    order = np.argsort(dst, kind="stable")
    cnt = np.bincount(blk_of_edge, minlength=n_blocks_total)
    K = max(2, int(math.ceil(cnt.max() / P)))
    C = B * K
    Ep = C * P

    sorted_eids = order
    sorted_blk = blk_of_edge[order]
    startpos = np.zeros(n_blocks_total + 1, np.int64)
    np.cumsum(cnt, out=startpos[1:])
    pos_in_blk = np.arange(E) - startpos[sorted_blk]

    core_of = sorted_blk // B
    blk_local = sorted_blk % B
    slot = blk_local * (K * P) + pos_in_blk

    src_sh = np.zeros((n_cores, Ep), np.int32)
    dstc_sh = np.zeros((n_cores, Ep), np.float16)
    amask_sh = np.zeros((n_cores, Ep), np.float32)
    ea_sh = np.zeros((n_cores, Ep, EA), np.float16)
    eid_sh = np.full((n_cores, Ep), -1, np.int64)

    ea16 = np.asarray(edge_attr, np.float16)
    for c in range(n_cores):
        m = core_of == c
        s = slot[m]
        eids = sorted_eids[m]
        src_sh[c, s] = src[eids]
        dstc_sh[c, s] = (dst[eids] % P).astype(np.float16)
        amask_sh[c, s] = 1.0
        ea_sh[c, s] = ea16[eids]
        eid_sh[c, s] = eids

    xsp = np.concatenate([np.asarray(x, np.float32),
                          np.asarray(splitter_probs, np.float32)], axis=1)
    xsp_p = np.zeros((Np, XS), np.float32)
    xsp_p[:N] = xsp
    gids_p = np.zeros(Np, np.float32)
    gids_p[:N] = gids.astype(np.float32)

    shards = []
    for c in range(n_cores):
        lo = c * NLp
        shards.append(dict(
            off_src=np.ascontiguousarray(src_sh[c].reshape(C, P).T),      # [128, C] i32
            dstcol=np.ascontiguousarray(dstc_sh[c].reshape(C, P).T),      # [128, C] f16
            amask=np.ascontiguousarray(amask_sh[c].reshape(C, P).T),      # [128, C] f32
            eaT=np.ascontiguousarray(ea_sh[c].reshape(Ep, EA).T),         # [4, Ep] f16
            xspT=np.ascontiguousarray(xsp_p[lo:lo + NLp].T),              # [7, NLp] f32
            gidcol=np.ascontiguousarray(
                gids_p[lo:lo + NLp].reshape(B, P).T),                     # [128, B] f32
        ))

    meta = dict(N=N, E=E, Np=Np, NLp=NLp, B=B, K=K, C=C, Ep=Ep, eid_sh=eid_sh)
    return shards, meta


def _pack_rows(W, dtype):
    """[R, X] -> [128, ceil(R/128), X] partition-chunked, zero padded."""
    W = np.asarray(W, dtype)
    R, X = W.shape
    nck = int(math.ceil(R / P))
    out = np.zeros((P, nck, X), dtype)
    for ci in range(nck):
        r0 = ci * P
        rl = min(P, R - r0)
        out[:rl, ci] = W[r0:r0 + rl]
    return out


def _host_weights(group_probs, endpoint_preds,
                  W_in, b_in, Wq, Wk, Wv, We, Wo, bo, ln1_g, ln1_b,
                  W_ff1, b_ff1, W_ff2, b_ff2, ln2_g, ln2_b,
                  W_e1, b_e1, W_e2, b_e2, W_e3, b_e3):
    f32, f16 = np.float32, np.float16

    def bc(v, X):
        return np.broadcast_to(np.asarray(v, f32)[None, :], (P, X)).copy()

    w = {}
    W_in = np.asarray(W_in, f32)
    w["Wxs"] = np.ascontiguousarray(np.concatenate([W_in[0:4], W_in[7:10]], axis=0))
    w["Wgp"] = np.ascontiguousarray(W_in[4:7])
    w["Wep"] = np.ascontiguousarray(W_in[10:16])
    w["gpT"] = np.ascontiguousarray(np.asarray(group_probs, f32).T)
    w["epT"] = np.ascontiguousarray(np.asarray(endpoint_preds, f32).T)
    w["b_in"] = bc(b_in, HID)

    scale = f32(1.0 / np.sqrt(DH))
    for l in range(L):
        w[f"Wq{l}"] = _pack_rows(np.asarray(Wq[l], f32) * scale, f16)       # [128,2,192]
        w[f"WkWv{l}"] = _pack_rows(np.concatenate(
            [np.asarray(Wk[l], f32), np.asarray(Wv[l], f32)], axis=1), f16)  # [128,2,384]
        w[f"We2_{l}"] = np.concatenate(
            [np.asarray(We[l], f16), np.asarray(We[l], f16)], axis=1)        # [4,384]
        w[f"Wo{l}"] = _pack_rows(Wo[l], f16)
        w[f"bo{l}"] = bc(bo[l], HID)
        w[f"ln1g{l}"] = bc(ln1_g[l], HID)
        w[f"ln1b{l}"] = bc(ln1_b[l], HID)
        w[f"Wff1_{l}"] = _pack_rows(W_ff1[l], f16)                           # [128,2,384]
        w[f"bff1_{l}"] = bc(b_ff1[l], FFN)
        w[f"Wff2_{l}"] = _pack_rows(W_ff2[l], f16)                           # [128,3,192]
        w[f"bff2_{l}"] = bc(b_ff2[l], HID)
        w[f"ln2g{l}"] = bc(ln2_g[l], HID)
        w[f"ln2b{l}"] = bc(ln2_b[l], HID)

    W_e1 = np.asarray(W_e1, f32)
    w["W1a"] = _pack_rows(W_e1[0:HID], f16)
    w["W1b"] = _pack_rows(W_e1[HID:2 * HID], f16)
    w["W1c"] = np.asarray(W_e1[2 * HID:], f16)
    w["b_e1"] = bc(b_e1, HID)
    w["W2"] = _pack_rows(W_e2, f16)                                          # [128,2,96]
    w["W3"] = np.asarray(W_e3, f16)                                          # [96,1]
    w["c_iota_row"] = np.broadcast_to(np.arange(P, dtype=f16)[None, :], (P, P)).copy()
    w["c_iota_col4"] = np.ascontiguousarray(
        (np.arange(G_TBL, dtype=f32).reshape(G_TBL // P, P).T)[:, :, None]
        * np.ones((1, 1, P), f32))
    b_e3c = float(np.asarray(b_e3, f32).reshape(-1)[0])
    return w, b_e3c


# ----------------------------------------------------------------------------
# device program
# ----------------------------------------------------------------------------

def build_program(meta, b_e3_const, n_cores, debug=False):
    B, K, C, Ep, NLp, Np = (meta["B"], meta["K"], meta["C"], meta["Ep"],
                            meta["NLp"], meta["Np"])
    FC = [(0, P), (P, HID - P)]   # feature chunks of 192

    nc = bass.Bass()

    def param(name, shape, dtype):
        return nc.declare_dram_parameter(name, list(shape), dtype, isOutput=False)

    off_src = param("off_src", [P, C], dt.int32)
    dstcol = param("dstcol", [P, C], dt.float16)
    amask = param("amask", [P, C], dt.float32)
    eaT = param("eaT", [EA, Ep], dt.float16)
    xspT = param("xspT", [XS, NLp], dt.float32)
    gidcol = param("gidcol", [P, B], dt.float32)
    Wxs = param("Wxs", [XS, HID], dt.float32)
    Wgp = param("Wgp", [GP, HID], dt.float32)
    Wep = param("Wep", [EP, HID], dt.float32)
    gpT = param("gpT", [GP, G_TBL], dt.float32)
    epT = param("epT", [EP, G_TBL], dt.float32)
    b_in = param("b_in", [P, HID], dt.float32)
    Wq_p = [param(f"Wq{l}", [P, 2, HID], dt.float16) for l in range(L)]
    WkWv_p = [param(f"WkWv{l}", [P, 2, H2], dt.float16) for l in range(L)]
    We2_p = [param(f"We2_{l}", [EA, H2], dt.float16) for l in range(L)]
    Wo_p = [param(f"Wo{l}", [P, 2, HID], dt.float16) for l in range(L)]
    bo_p = [param(f"bo{l}", [P, HID], dt.float32) for l in range(L)]
    ln1g_p = [param(f"ln1g{l}", [P, HID], dt.float32) for l in range(L)]
    ln1b_p = [param(f"ln1b{l}", [P, HID], dt.float32) for l in range(L)]
    Wff1_p = [param(f"Wff1_{l}", [P, 2, FFN], dt.float16) for l in range(L)]
    bff1_p = [param(f"bff1_{l}", [P, FFN], dt.float32) for l in range(L)]
    Wff2_p = [param(f"Wff2_{l}", [P, 3, HID], dt.float16) for l in range(L)]
    bff2_p = [param(f"bff2_{l}", [P, HID], dt.float32) for l in range(L)]
    ln2g_p = [param(f"ln2g{l}", [P, HID], dt.float32) for l in range(L)]
    ln2b_p = [param(f"ln2b{l}", [P, HID], dt.float32) for l in range(L)]
    W1a = param("W1a", [P, 2, HID], dt.float16)
    W1b = param("W1b", [P, 2, HID], dt.float16)
    W1c = param("W1c", [EA, HID], dt.float16)
    b_e1 = param("b_e1", [P, HID], dt.float32)
    W2 = param("W2", [P, 2, Z2], dt.float16)
    W3 = param("W3", [Z2, 1], dt.float16)
    c_iota_row = param("c_iota_row", [P, P], dt.float16)
    c_iota_col4 = param("c_iota_col4", [P, G_TBL // P, P], dt.float32)

    out_z = nc.declare_dram_parameter("out_z", [1, Ep], dt.float32, isOutput=True)
    if debug:
        dbg_h0 = nc.declare_dram_parameter("dbg_h0", [P, B, HID], dt.float32, isOutput=True)
        dbg_h1 = nc.declare_dram_parameter("dbg_h1", [P, B, HID], dt.float32, isOutput=True)
        dbg_msg = nc.declare_dram_parameter("dbg_msg", [P, B, HID], dt.float32, isOutput=True)
        dbg_q = nc.declare_dram_parameter("dbg_q", [P, B, HID], dt.float32, isOutput=True)
        dbg_kvt = nc.declare_dram_parameter("dbg_kvt", [Np, H2], dt.float32, isOutput=True)
        dbg_kvg = nc.declare_dram_parameter("dbg_kvg", [P, 8, H2], dt.float32, isOutput=True)
        dbg_logit = nc.declare_dram_parameter("dbg_logit", [P, 8, HEADS], dt.float32, isOutput=True)
        dbg_S = nc.declare_dram_parameter("dbg_S", [P, 8, P], dt.float32, isOutput=True)

    kv_loc = nc.dram_tensor("kv_loc", [NLp, H2], dt.float16)
    kv_tbl = nc.dram_tensor("kv_tbl", [Np, H2], dt.float16)
    u_loc = nc.dram_tensor("u_loc", [NLp, HID], dt.float16)
    u_tbl = nc.dram_tensor("u_tbl", [Np, HID], dt.float16)

    GRP = 4  # z3 chunks per output DMA

    from contextlib import ExitStack
    with tile.TileContext(nc) as tc:
        with tc.tile_pool(name="pers", bufs=1) as pers, \
             tc.tile_pool(name="wp", bufs=1) as wpool, \
             tc.tile_pool(name="ps", bufs=5, space="PSUM") as ps, \
             tc.tile_pool(name="psB", bufs=2, space="PSUM") as psB:
            _setup_stack = ExitStack()
            sup = _setup_stack.enter_context(tc.tile_pool(name="setup", bufs=1))
            sb = sup  # setup phase allocations go to the scoped pool

            # ---------------- persistent state ----------------
            h_loc = pers.tile([P, B, HID], dt.float32)
            hT0 = pers.tile([P, NLp], dt.float16)
            hT1 = pers.tile([HID - P, NLp], dt.float16)
            q_loc = pers.tile([P, B, HID], dt.float16)
            w_loc = pers.tile([P, B, HID], dt.float16)
            msg_loc = pers.tile([P, B, HID], dt.float32)

            ident32 = pers.tile([P, P], dt.float32)
            make_identity(nc, ident32[:])
            ident16 = pers.tile([P, P], dt.float16)
            nc.vector.tensor_copy(out=ident16[:], in_=ident32[:])
            iota_row16 = pers.tile([P, P], dt.float16)
            nc.sync.dma_start(out=iota_row16[:], in_=c_iota_row[:, :])
            iotag_t = sup.tile([P, G_TBL // P, P], dt.float32)
            nc.sync.dma_start(out=iotag_t[:], in_=c_iota_col4[:, :, :])
            iotag = [iotag_t[:, gc, :] for gc in range(G_TBL // P)]

            offs_t = pers.tile([P, C], dt.int32)
            nc.sync.dma_start(out=offs_t[:], in_=off_src[:, :])
            dstc_t = pers.tile([P, C], dt.float16)
            nc.sync.dma_start(out=dstc_t[:], in_=dstcol[:, :])
            amask_t = pers.tile([P, C], dt.float32)
            nc.sync.dma_start(out=amask_t[:], in_=amask[:, :])
            xspT_t = sup.tile([XS, NLp], dt.float32)
            nc.sync.dma_start(out=xspT_t[:], in_=xspT[:, :])
            gid_t = sup.tile([P, B], dt.float32)
            nc.sync.dma_start(out=gid_t[:], in_=gidcol[:, :])

            def wtile(pp, shape, dtype, tag):
                t_ = wpool.tile(list(shape), dtype, tag=tag)
                nc.sync.dma_start(out=t_[:], in_=pp[...])
                return t_

            Wxs_t = wtile(Wxs, [XS, HID], dt.float32, "Wxs")
            Wgp_t = wtile(Wgp, [GP, HID], dt.float32, "Wgp")
            Wep_t = wtile(Wep, [EP, HID], dt.float32, "Wep")
            gpT_t = wtile(gpT, [GP, G_TBL], dt.float32, "gpT")
            epT_t = wtile(epT, [EP, G_TBL], dt.float32, "epT")
            b_in_t = wtile(b_in, [P, HID], dt.float32, "b_in")

            # ---------------- T12 [512, 192] f32 ----------------
            T12 = sup.tile([P, G_TBL // P, HID], dt.float32)
            for gc in range(G_TBL // P):
                pt = ps.tile([P, HID], dt.float32, space="PSUM", tag="mm")
                nc.tensor.matmul(out=pt[:], lhsT=gpT_t[:, gc * P:(gc + 1) * P],
                                 rhs=Wgp_t[:], start=True, stop=False)
                nc.tensor.matmul(out=pt[:], lhsT=epT_t[:, gc * P:(gc + 1) * P],
                                 rhs=Wep_t[:], start=False, stop=True)
                nc.vector.tensor_copy(out=T12[:, gc, :], in_=pt[:])

            # ---------------- h0 ----------------
            for b in range(B):
                gbc_ps = ps.tile([P, P], dt.float32, space="PSUM", tag="mm")
                nc.tensor.transpose(out=gbc_ps[:], in_=gid_t[:, b:b + 1].to_broadcast([P, P]),
                                    identity=ident32[:])
                gbc = sb.tile([P, P], dt.float32, tag="gbc_s")
                nc.scalar.copy(out=gbc[:], in_=gbc_ps[:])
                hp = psB.tile([P, HID], dt.float32, space="PSUM", tag="acc")
                nc.tensor.matmul(out=hp[:], lhsT=xspT_t[:, b * P:(b + 1) * P],
                                 rhs=Wxs_t[:], start=True, stop=False)
                for gc in range(G_TBL // P):
                    og = sb.tile([P, P], dt.float32, tag="og")
                    nc.vector.tensor_tensor(out=og[:], in0=iotag[gc], in1=gbc[:],
                                            op=mybir.AluOpType.is_equal)
                    nc.tensor.matmul(out=hp[:], lhsT=og[:], rhs=T12[:, gc, :],
                                     start=False, stop=(gc == G_TBL // P - 1))
                nc.vector.tensor_add(out=h_loc[:, b, :], in0=hp[:], in1=b_in_t[:])

            if debug:
                for b in range(B):
                    t_ = sb.tile([P, HID], dt.float32, tag="dbgc")
                    nc.vector.tensor_copy(out=t_[:], in_=h_loc[:, b, :])
                    nc.sync.dma_start(out=dbg_h0[:, b, :], in_=t_[:])
            _setup_stack.close()
            _work_stack = ExitStack()
            sb = _work_stack.enter_context(tc.tile_pool(name="sbN", bufs=2))
            sbE = _work_stack.enter_context(tc.tile_pool(name="sbE", bufs=3))
            gat = _work_stack.enter_context(tc.tile_pool(name="gat", bufs=6))

            # ---------------- helpers ----------------
            def transpose_h(b):
                for ci, (f0, fl) in enumerate(FC):
                    tp = ps.tile([P, P], dt.float32, space="PSUM", tag="mm")
                    nc.tensor.transpose(out=tp[:fl, :], in_=h_loc[:, b, f0:f0 + fl],
                                        identity=ident32[:])
                    dstt = hT0 if ci == 0 else hT1
                    nc.scalar.copy(out=dstt[:fl, b * P:(b + 1) * P], in_=tp[:fl, :])

            def layer_norm(b, g_t, b_t):
                red = sb.tile([P, 1], dt.float32, tag="ln_m")
                nc.vector.tensor_reduce(out=red[:], in_=h_loc[:, b, :],
                                        axis=mybir.AxisListType.X, op=mybir.AluOpType.add)
                m = sb.tile([P, 1], dt.float32, tag="ln_mm")
                nc.vector.tensor_scalar_mul(out=m[:], in0=red[:], scalar1=1.0 / HID)
                xc = sb.tile([P, HID], dt.float32, tag="ln_xc")
                nc.vector.tensor_scalar_sub(out=xc[:], in0=h_loc[:, b, :], scalar1=m[:, 0:1])
                prod = sb.tile([P, HID], dt.float32, tag="ln_p")
                nc.vector.tensor_tensor(out=prod[:], in0=xc[:], in1=xc[:],
                                        op=mybir.AluOpType.mult)
                sq = sb.tile([P, 1], dt.float32, tag="ln_sq")
                nc.vector.tensor_reduce(out=sq[:], in_=prod[:],
                                        axis=mybir.AxisListType.X, op=mybir.AluOpType.add)
                var = sb.tile([P, 1], dt.float32, tag="ln_v")
                nc.vector.tensor_scalar(out=var[:], in0=sq[:], scalar1=1.0 / HID,
                                        scalar2=1e-5, op0=mybir.AluOpType.mult,
                                        op1=mybir.AluOpType.add)
                rv = sb.tile([P, 1], dt.float32, tag="ln_r")
                nc.vector.reciprocal(out=rv[:], in_=var[:])
                rs = sb.tile([P, 1], dt.float32, tag="ln_rs")
                nc.scalar.sqrt(out=rs[:], in_=rv[:])
                nc.vector.tensor_scalar_mul(out=xc[:], in0=xc[:], scalar1=rs[:, 0:1])
                nc.vector.tensor_tensor(out=xc[:], in0=xc[:], in1=g_t[:],
                                        op=mybir.AluOpType.mult)
                nc.vector.tensor_tensor(out=h_loc[:, b, :], in0=xc[:], in1=b_t[:],
                                        op=mybir.AluOpType.add)

            def edge_common(t, b):
                """Gather + one-hot S/S^T for chunk t (dst-block b)."""
                S = sbE.tile([P, P], dt.float16, tag="S")
                nc.vector.tensor_tensor(out=S[:], in0=dstc_t[:, t:t + 1].to_broadcast([P, P]),
                                        in1=iota_row16[:], op=mybir.AluOpType.is_equal)
                Stp = ps.tile([P, P], dt.float16, space="PSUM", tag="mm")
                nc.tensor.transpose(out=Stp[:], in_=S[:], identity=ident16[:])
                St = sbE.tile([P, P], dt.float16, tag="St")
                nc.scalar.copy(out=St[:], in_=Stp[:])
                return S, St

            # ---------------- layers ----------------
            for l in range(L):
                Wq_t = wtile(Wq_p[l], [P, 2, HID], dt.float16, "Wq")
                WkWv_t = wtile(WkWv_p[l], [P, 2, H2], dt.float16, "WkWv")
                We2_t = wtile(We2_p[l], [EA, H2], dt.float16, "We2")
                Wo_t = wtile(Wo_p[l], [P, 2, HID], dt.float16, "Wo")
                bo_t = wtile(bo_p[l], [P, HID], dt.float32, "bo")
                ln1g_t = wtile(ln1g_p[l], [P, HID], dt.float32, "ln1g")
                ln1b_t = wtile(ln1b_p[l], [P, HID], dt.float32, "ln1b")
                Wff1_t = wtile(Wff1_p[l], [P, 2, FFN], dt.float16, "Wff1")
                bff1_t = wtile(bff1_p[l], [P, FFN], dt.float32, "bff1")
                Wff2_t = wtile(Wff2_p[l], [P, 3, HID], dt.float16, "Wff2")
                bff2_t = wtile(bff2_p[l], [P, HID], dt.float32, "bff2")
                ln2g_t = wtile(ln2g_p[l], [P, HID], dt.float32, "ln2g")
                ln2b_t = wtile(ln2b_p[l], [P, HID], dt.float32, "ln2b")

                for b in range(B):
                    transpose_h(b)

                for b in range(B):
                    qp = ps.tile([P, HID], dt.float32, space="PSUM", tag="mm")
                    kvp = ps.tile([P, H2], dt.float32, space="PSUM", tag="mm")
                    for ci, (f0, fl) in enumerate(FC):
                        hTt = hT0 if ci == 0 else hT1
                        lhs = hTt[:fl, b * P:(b + 1) * P]
                        nc.tensor.matmul(out=qp[:], lhsT=lhs, rhs=Wq_t[:fl, ci, :],
                                         start=(ci == 0), stop=(ci == 1))
                        nc.tensor.matmul(out=kvp[:], lhsT=lhs, rhs=WkWv_t[:fl, ci, :],
                                         start=(ci == 0), stop=(ci == 1))
                    nc.scalar.copy(out=q_loc[:, b, :], in_=qp[:])
                    kvf = sb.tile([P, H2], dt.float16, tag="kvf")
                    nc.scalar.copy(out=kvf[:], in_=kvp[:])
                    nc.sync.dma_start(out=kv_loc[b * P:(b + 1) * P, :], in_=kvf[:])

                nc.gpsimd.collective_compute(
                    "AllGather", mybir.AluOpType.bypass,
                    replica_groups=[list(range(n_cores))],
                    ins=[kv_loc[:, :]], outs=[kv_tbl[:, :]])
                if debug and l == 0:
                    for b in range(B):
                        tq = sb.tile([P, HID], dt.float32, tag="dbgc")
                        nc.vector.tensor_copy(out=tq[:], in_=q_loc[:, b, :])
                        nc.sync.dma_start(out=dbg_q[:, b, :], in_=tq[:])
                    for bb_ in range(Np // P):
                        tk = sb.tile([P, H2], dt.float32, tag="dbgk")
                        tk16 = sb.tile([P, H2], dt.float16, tag="dbgk16")
                        nc.sync.dma_start(out=tk16[:], in_=kv_tbl[bb_ * P:(bb_ + 1) * P, :])
                        nc.vector.tensor_copy(out=tk[:], in_=tk16[:])
                        nc.sync.dma_start(out=dbg_kvt[bb_ * P:(bb_ + 1) * P, :], in_=tk[:])

                for b in range(B):
                    ea_blk = sb.tile([EA, K * P], dt.float16, tag="ea_blk")
                    nc.sync.dma_start(out=ea_blk[:], in_=eaT[:, b * K * P:(b + 1) * K * P])
                    acc = psB.tile([P, HEADS + HID], dt.float32, space="PSUM", tag="acc")
                    for kk in range(K):
                        t = b * K + kk
                        kvg = gat.tile([P, H2], dt.float16, tag="kvg")
                        nc.gpsimd.indirect_dma_start(
                            out=kvg[:], out_offset=None, in_=kv_tbl[:, :],
                            in_offset=bass.IndirectOffsetOnAxis(ap=offs_t[:, t:t + 1], axis=0))
                        S, St = edge_common(t, b)
                        qd = ps.tile([P, HID], dt.float32, space="PSUM", tag="mm")
                        nc.tensor.matmul(out=qd[:], lhsT=St[:], rhs=q_loc[:, b, :],
                                         start=True, stop=True)
                        ep_ = ps.tile([P, H2], dt.float32, space="PSUM", tag="mm")
                        nc.tensor.matmul(out=ep_[:], lhsT=ea_blk[:, kk * P:(kk + 1) * P],
                                         rhs=We2_t[:], start=True, stop=True)
                        if debug and l == 0 and t < 8:
                            tg = sb.tile([P, H2], dt.float32, tag="dbgk")
                            nc.vector.tensor_copy(out=tg[:], in_=kvg[:])
                            nc.sync.dma_start(out=dbg_kvg[:, t, :], in_=tg[:])
                            tS = sb.tile([P, P], dt.float32, tag="dbgc")
                            nc.vector.tensor_copy(out=tS[:], in_=S[:])
                            nc.sync.dma_start(out=dbg_S[:, t, :], in_=tS[:])
                        kvs = sbE.tile([P, H2], dt.float16, tag="kvs")
                        nc.vector.tensor_tensor(out=kvs[:], in0=kvg[:], in1=ep_[:],
                                                op=mybir.AluOpType.add)
                        prod = sbE.tile([P, HID], dt.float32, tag="prod")
                        nc.vector.tensor_tensor(out=prod[:], in0=qd[:], in1=kvs[:, 0:HID],
                                                op=mybir.AluOpType.mult)
                        logit = sbE.tile([P, HEADS], dt.float32, tag="logit")
                        nc.vector.tensor_reduce(
                            out=logit[:], in_=prod[:].rearrange("p (h d) -> p h d", h=HEADS),
                            axis=mybir.AxisListType.X, op=mybir.AluOpType.add)
                        if debug and l == 0 and t < 8:
                            nc.sync.dma_start(out=dbg_logit[:, t, :], in_=logit[:])
                        combo = sbE.tile([P, HEADS + HID], dt.float16, tag="combo")
                        ae = sbE.tile([P, HEADS], dt.float16, tag="ae")
                        nc.scalar.activation(out=ae[:], in_=logit[:],
                                             func=mybir.ActivationFunctionType.Exp)
                        nc.vector.tensor_scalar_mul(out=combo[:, 0:HEADS], in0=ae[:],
                                                    scalar1=amask_t[:, t:t + 1])
                        nc.vector.tensor_tensor(
                            out=combo[:, HEADS:].rearrange("p (h d) -> p h d", h=HEADS),
                            in0=kvs[:, HID:].rearrange("p (h d) -> p h d", h=HEADS),
                            in1=combo[:, 0:HEADS].rearrange("p (h o) -> p h o", o=1)
                                .to_broadcast([P, HEADS, DH]),
                            op=mybir.AluOpType.mult)
                        nc.tensor.matmul(out=acc[:], lhsT=S[:], rhs=combo[:],
                                         start=(kk == 0), stop=(kk == K - 1),
                                         skip_group_check=True)
                    den = sbE.tile([P, HEADS], dt.float32, tag="den")
                    nc.vector.tensor_scalar_add(out=den[:], in0=acc[:, 0:HEADS],
                                                scalar1=1e-16)
                    rden = sbE.tile([P, HEADS], dt.float32, tag="rden")
                    nc.vector.reciprocal(out=rden[:], in_=den[:])
                    nc.vector.tensor_tensor(
                        out=msg_loc[:, b, :].rearrange("p (h d) -> p h d", h=HEADS),
                        in0=acc[:, HEADS:].rearrange("p (h d) -> p h d", h=HEADS),
                        in1=rden[:].rearrange("p (h o) -> p h o", o=1)
                            .to_broadcast([P, HEADS, DH]),
                        op=mybir.AluOpType.mult)

                if debug and l == 0:
                    for b in range(B):
                        tm = sb.tile([P, HID], dt.float32, tag="dbgc")
                        nc.vector.tensor_copy(out=tm[:], in_=msg_loc[:, b, :])
                        nc.sync.dma_start(out=dbg_msg[:, b, :], in_=tm[:])

                # node update: h = LN1(h + msg@Wo + bo)
                for b in range(B):
                    mT0 = sb.tile([P, P], dt.float16, tag="mT0")
                    mT1 = sb.tile([HID - P, P], dt.float16, tag="mT1")
                    for ci, (f0, fl) in enumerate(FC):
                        tp = ps.tile([P, P], dt.float32, space="PSUM", tag="mm")
                        nc.tensor.transpose(out=tp[:fl, :], in_=msg_loc[:, b, f0:f0 + fl],
                                            identity=ident32[:])
                        nc.scalar.copy(out=(mT0 if ci == 0 else mT1)[:fl, :], in_=tp[:fl, :])
                    yp = ps.tile([P, HID], dt.float32, space="PSUM", tag="mm")
                    for ci, (f0, fl) in enumerate(FC):
                        nc.tensor.matmul(out=yp[:], lhsT=(mT0 if ci == 0 else mT1)[:fl, :],
                                         rhs=Wo_t[:fl, ci, :], start=(ci == 0), stop=(ci == 1))
                    nc.vector.tensor_tensor(out=h_loc[:, b, :], in0=h_loc[:, b, :],
                                            in1=yp[:], op=mybir.AluOpType.add)
                    nc.vector.tensor_tensor(out=h_loc[:, b, :], in0=h_loc[:, b, :],
                                            in1=bo_t[:], op=mybir.AluOpType.add)
                    layer_norm(b, ln1g_t, ln1b_t)

                if debug and l == 0:
                    for b in range(B):
                        th = sb.tile([P, HID], dt.float32, tag="dbgc")
                        nc.vector.tensor_copy(out=th[:], in_=h_loc[:, b, :])
                        nc.sync.dma_start(out=dbg_h1[:, b, :], in_=th[:])
                # FFN
                for b in range(B):
                    transpose_h(b)
                for b in range(B):
                    f1p = ps.tile([P, FFN], dt.float32, space="PSUM", tag="mm")
                    for ci, (f0, fl) in enumerate(FC):
                        hTt = hT0 if ci == 0 else hT1
                        nc.tensor.matmul(out=f1p[:], lhsT=hTt[:fl, b * P:(b + 1) * P],
                                         rhs=Wff1_t[:fl, ci, :], start=(ci == 0), stop=(ci == 1))
                    f1 = sb.tile([P, FFN], dt.float32, tag="f1")
                    nc.vector.tensor_tensor(out=f1[:], in0=f1p[:], in1=bff1_t[:],
                                            op=mybir.AluOpType.add)
                    f1r = sb.tile([P, FFN], dt.float16, tag="f1r")
                    nc.scalar.activation(out=f1r[:], in_=f1[:],
                                         func=mybir.ActivationFunctionType.Relu)
                    f1T = sb.tile([P, 3, P], dt.float16, tag="f1T")
                    for ci in range(3):
                        tp16 = ps.tile([P, P], dt.float16, space="PSUM", tag="mm")
                        nc.tensor.transpose(out=tp16[:], in_=f1r[:, ci * P:(ci + 1) * P],
                                            identity=ident16[:])
                        nc.scalar.copy(out=f1T[:, ci, :], in_=tp16[:])
                    f2p = ps.tile([P, HID], dt.float32, space="PSUM", tag="mm")
                    for ci in range(3):
                        nc.tensor.matmul(out=f2p[:], lhsT=f1T[:, ci, :],
                                         rhs=Wff2_t[:, ci, :], start=(ci == 0), stop=(ci == 2))
                    nc.vector.tensor_tensor(out=h_loc[:, b, :], in0=h_loc[:, b, :],
                                            in1=f2p[:], op=mybir.AluOpType.add)
                    nc.vector.tensor_tensor(out=h_loc[:, b, :], in0=h_loc[:, b, :],
                                            in1=bff2_t[:], op=mybir.AluOpType.add)
                    layer_norm(b, ln2g_t, ln2b_t)

            # ---------------- edge head ----------------
            W1a_t = wtile(W1a, [P, 2, HID], dt.float16, "W1a")
            W1b_t = wtile(W1b, [P, 2, HID], dt.float16, "W1b")
            W1c_t = wtile(W1c, [EA, HID], dt.float16, "W1c")
            b_e1_t = wtile(b_e1, [P, HID], dt.float32, "be1")
            W2_t = wtile(W2, [P, 2, Z2], dt.float16, "W2")
            W3_t = wtile(W3, [Z2, 1], dt.float16, "W3")

            for b in range(B):
                transpose_h(b)
            for b in range(B):
                up = ps.tile([P, HID], dt.float32, space="PSUM", tag="mm")
                wp_ = ps.tile([P, HID], dt.float32, space="PSUM", tag="mm")
                for ci, (f0, fl) in enumerate(FC):
                    hTt = hT0 if ci == 0 else hT1
                    lhs = hTt[:fl, b * P:(b + 1) * P]
                    nc.tensor.matmul(out=up[:], lhsT=lhs, rhs=W1a_t[:fl, ci, :],
                                     start=(ci == 0), stop=(ci == 1))
                    nc.tensor.matmul(out=wp_[:], lhsT=lhs, rhs=W1b_t[:fl, ci, :],
                                     start=(ci == 0), stop=(ci == 1))
                uf = sb.tile([P, HID], dt.float32, tag="uf")
                nc.vector.tensor_tensor(out=uf[:], in0=up[:], in1=b_e1_t[:],
                                        op=mybir.AluOpType.add)
                uf16 = sb.tile([P, HID], dt.float16, tag="uf16")
                nc.vector.tensor_copy(out=uf16[:], in_=uf[:])
                nc.sync.dma_start(out=u_loc[b * P:(b + 1) * P, :], in_=uf16[:])
                nc.scalar.copy(out=w_loc[:, b, :], in_=wp_[:])

            nc.gpsimd.collective_compute(
                "AllGather", mybir.AluOpType.bypass,
                replica_groups=[list(range(n_cores))],
                ins=[u_loc[:, :]], outs=[u_tbl[:, :]])

            z3buf = None
            for b in range(B):
                ea_blk = sb.tile([EA, K * P], dt.float16, tag="ea_blk")
                nc.sync.dma_start(out=ea_blk[:], in_=eaT[:, b * K * P:(b + 1) * K * P])
                for kk in range(K):
                    t = b * K + kk
                    ug = gat.tile([P, HID], dt.float16, tag="ug")
                    nc.gpsimd.indirect_dma_start(
                        out=ug[:], out_offset=None, in_=u_tbl[:, :],
                        in_offset=bass.IndirectOffsetOnAxis(ap=offs_t[:, t:t + 1], axis=0))
                    S, St = edge_common(t, b)
                    z1p = ps.tile([P, HID], dt.float32, space="PSUM", tag="mm")
                    nc.tensor.matmul(out=z1p[:], lhsT=St[:], rhs=w_loc[:, b, :],
                                     start=True, stop=False)
                    nc.tensor.matmul(out=z1p[:], lhsT=ea_blk[:, kk * P:(kk + 1) * P],
                                     rhs=W1c_t[:], start=False, stop=True)
                    z1s = sb.tile([P, HID], dt.float32, tag="z1s")
                    nc.vector.tensor_tensor(out=z1s[:], in0=z1p[:], in1=ug[:],
                                            op=mybir.AluOpType.add)
                    z1r = sb.tile([P, HID], dt.float16, tag="z1r")
                    nc.scalar.activation(out=z1r[:], in_=z1s[:],
                                         func=mybir.ActivationFunctionType.Relu)
                    z1T0 = sb.tile([P, P], dt.float16, tag="z1T0")
                    z1T1 = sb.tile([HID - P, P], dt.float16, tag="z1T1")
                    for ci, (f0, fl) in enumerate(FC):
                        tp16 = ps.tile([P, P], dt.float16, space="PSUM", tag="mm")
                        nc.tensor.transpose(out=tp16[:fl, :], in_=z1r[:, f0:f0 + fl],
                                            identity=ident16[:])
                        nc.scalar.copy(out=(z1T0 if ci == 0 else z1T1)[:fl, :], in_=tp16[:fl, :])
                    z2p = ps.tile([Z2, P], dt.float32, space="PSUM", tag="mm")
                    for ci, (f0, fl) in enumerate(FC):
                        nc.tensor.matmul(out=z2p[:], lhsT=W2_t[:fl, ci, :],
                                         rhs=(z1T0 if ci == 0 else z1T1)[:fl, :],
                                         start=(ci == 0), stop=(ci == 1))
                    z2r = sb.tile([Z2, P], dt.float16, tag="z2r")
                    nc.scalar.activation(out=z2r[:], in_=z2p[:],
                                         func=mybir.ActivationFunctionType.Relu)
                    z3p = ps.tile([1, P], dt.float32, space="PSUM", tag="mm")
                    nc.tensor.matmul(out=z3p[:], lhsT=W3_t[:, :], rhs=z2r[:],
                                     start=True, stop=True)
                    if t % GRP == 0:
                        z3buf = sb.tile([1, GRP * P], dt.float32, tag="z3b")
                    nc.scalar.activation(out=z3buf[:, (t % GRP) * P:(t % GRP + 1) * P],
                                         in_=z3p[:],
                                         func=mybir.ActivationFunctionType.Copy,
                                         bias=float(b_e3_const))
                    if t % GRP == GRP - 1:
                        g0 = (t // GRP) * GRP * P
                        nc.sync.dma_start(out=out_z[:, g0:g0 + GRP * P], in_=z3buf[:])
            if C % GRP != 0:
                # flush tail group
                done = (C // GRP) * GRP
                rem = C - done
                nc.sync.dma_start(out=out_z[:, done * P:C * P],
                                  in_=z3buf[:, 0:rem * P])
            _work_stack.close()

    return nc


# ----------------------------------------------------------------------------
# public entry
# ----------------------------------------------------------------------------

def _run(inputs, n_cores, runner):
    shards, meta = _host_prep(
        inputs["x"], inputs["edge_index"], inputs["edge_attr"], inputs["batch"],
        inputs["group_ptr"], inputs["time_group_ids"], inputs["group_probs"],
        inputs["splitter_probs"], inputs["endpoint_preds"], n_cores)
    w, b_e3c = _host_weights(
        inputs["group_probs"], inputs["endpoint_preds"],
        *[inputs[k] for k in [
            "W_in", "b_in", "Wq", "Wk", "Wv", "We", "Wo", "bo",
            "ln1_g", "ln1_b", "W_ff1", "b_ff1", "W_ff2", "b_ff2",
            "ln2_g", "ln2_b", "W_e1", "b_e1", "W_e2", "b_e2", "W_e3", "b_e3"]])
    nc = build_program(meta, b_e3c, n_cores)
    in_maps = []
    for c in range(n_cores):
        m = dict(shards[c])
        m.update(w)
        in_maps.append(m)
    results = runner(nc, in_maps)
    E = meta["E"]
    out = np.zeros((E, 1), np.float32)
    for c in range(n_cores):
        z = np.asarray(results[c]["out_z"]).reshape(-1)
        eid = meta["eid_sh"][c]
        valid = eid >= 0
        out[eid[valid], 0] = z[valid]
    return out


def kernel(**inputs):
    from concourse.bass_utils import run_bass_kernel_spmd

    n_cores = 8

    def runner(nc, in_maps):
        split_excess_waits(nc, max_waits=1)
        br = run_bass_kernel_spmd(nc, in_maps, core_ids=list(range(n_cores)))
        return br.results

    return _run(inputs, n_cores, runner)
